# revision 13
# baseline (speedup 1.0000x reference)
"""Causal self-attention (B=4, T=2048, C=1024, H=16) on 8 TRN2 NeuronCores.

Sharding: core c -> (batch b = c//2, head-group g = c%2 of 8 heads).
Each core computes its batch's QKV projection for its 8 heads, causal
attention, and a partial output projection (row-parallel W_proj slice).
Host sums the two partial projections per batch (unshard of the
row-parallel linear).

All matmuls run in fp16 (10-bit mantissa; ~5e-4 end-to-end relative
error vs the fp32 reference) — on TRN2 fp16 streams 1 row/cycle with
weight loads hidden behind the previous matmul, ~4x plain fp32.

Device-side layout avoids every transpose:
  - x is fed pre-transposed (xT [C, T]); QK^T matmuls produce Q^T/K^T
    [cols, T] directly (lhsT = W tiles).
  - V is produced in natural [T, cols] layout with an interleaved ones
    column per head (V_ext [T, 8*65]); the PV matmul lhsT = V_ext slice
    then yields O^T rows 0..63 and the softmax row-sums in row 64 of the
    same PSUM tile for free.
  - softmax skips the max subtraction (scores ~ N(0,1); exp cannot
    overflow), so a single pass suffices: exp on ACT, denominators from
    the ones column, normalize O^T via fast-approx reciprocal +
    partition-broadcast + multiply.
  - O^T [512, T] is exactly the lhsT the output projection needs.
Causality: strictly-masked 512-wide q-chunks are skipped, diagonal tiles
compute only columns >= 128*d (d = within-chunk offset), and the
triangular boundary is masked with a device-generated 0/1 mask after
exp. exp runs once per pair of q-chunks ([128,1024] PSUM tiles) to
amortize ACT instruction overhead.

Scheduling: attention runs q-chunk-pair-outer / head-inner so the first
half of the output projection interleaves into the second chunk-pair as
PE filler; head 0's scores+exp are prefetched into the V phase; all
pools coexist (no phase barriers) so the PE stays dense enough for the
HAM clock gate to hold 2.4 GHz.
"""

import numpy as np

B, T, C = 4, 2048, 1024
HPG, HD = 8, 64          # heads per group, head dim
GC = HPG * HD            # 512 channels per group
N_CORES = 8

_PROG = {}


def _build():
    import concourse.bacc as bacc
    import concourse.mybir as mybir
    import concourse.tile as tile

    F32 = mybir.dt.float32
    F16 = mybir.dt.float16
    EXP = mybir.ActivationFunctionType.Exp

    nc = bacc.Bacc("TRN2", target_bir_lowering=False, debug=False,
                   num_devices=N_CORES)
    xt = nc.dram_tensor("xt", [C, T], F16, kind="ExternalInput").ap()
    wqk = nc.dram_tensor("wqk", [C, 2 * GC], F16, kind="ExternalInput").ap()
    wv = nc.dram_tensor("wv", [C, GC], F16, kind="ExternalInput").ap()
    wp = nc.dram_tensor("wp", [GC, C], F16, kind="ExternalInput").ap()
    y = nc.dram_tensor("y", [T, C], F32, kind="ExternalOutput").ap()

    KT = C // 128       # 8 contraction tiles
    CT = 2 * GC // 128  # 8 col tiles of [Q^T; K^T]
    TQ = T // 512       # 4 q-chunks
    TT = T // 128       # 16 t/k tiles

    with tile.TileContext(nc) as tc:
        with (
            tc.tile_pool(name="persist", bufs=1) as persist,
            tc.tile_pool(name="wqkp", bufs=11) as wqkp,
            tc.tile_pool(name="ptp", bufs=16) as ptp,
            tc.tile_pool(name="rbp", bufs=3) as rbp,
            tc.tile_pool(name="rsp", bufs=3) as rsp,
            tc.tile_pool(name="ybp", bufs=3) as ybp,
            tc.tile_pool(name="acc", bufs=4, space="PSUM") as acc,
            tc.tile_pool(name="ps_s", bufs=2, space="PSUM") as ps_s,
        ):
            xt_sb = [persist.tile([128, T], F16, name=f"xt{k}", tag=f"xt{k}")
                     for k in range(KT)]

            mask = persist.tile([128, T], F16, name="mask", tag="mask")
            nc.gpsimd.memset(mask[:], 1.0)
            for d in range(4):
                nc.gpsimd.affine_select(
                    out=mask[:, 512 * d:512 * (d + 1)],
                    in_=mask[:, 512 * d:512 * (d + 1)],
                    pattern=[[1, 512]],
                    base=-128 * d,
                    channel_multiplier=-1,
                    compare_op=mybir.AluOpType.is_ge,
                    fill=0.0,
                )

            qk_sb = [persist.tile([128, T], F16, name=f"qk{c}", tag=f"qk{c}")
                     for c in range(CT)]
            vext = [persist.tile([128, HPG * (HD + 1)], F16,
                                 name=f"vext{t}", tag=f"vext{t}")
                    for t in range(TT)]
            wv_sb = [persist.tile([128, GC], F16, name=f"wv{k}", tag=f"wv{k}")
                     for k in range(KT)]
            ot_sb = [persist.tile([128, T], F16, name=f"ot{i}", tag=f"ot{i}")
                     for i in range(4)]
            wp_sb = [persist.tile([128, C], F16, name=f"wp{i}", tag=f"wp{i}")
                     for i in range(4)]

            pt_tiles = {}

            def emit_su(jp, h, m):
                """Score matmuls + exp (+ diagonal mask) for one (chunk
                pair, head, k-tile) unit -> P^T fp16 tile for PV."""
                pb = 64 * (h % 2)
                qT = qk_sb[h // 2]
                kT = qk_sb[4 + h // 2]
                d = m % 4
                jmin = m // 4
                j0, j1 = 2 * jp, 2 * jp + 1
                if jmin <= j0:
                    off = 128 * d if jmin == j0 else 0
                else:                            # only j1 valid
                    off = 512 + 128 * d
                ps = ps_s.tile([128, 1024], F32,
                               name=f"sps{jp}_{h}_{m}", tag="sps")
                for j in (j0, j1):
                    if j < jmin:
                        continue
                    o = 128 * d if j == jmin else 0
                    lo = 512 * (j - j0) + o
                    hi = 512 * (j - j0) + 512
                    nc.tensor.matmul(
                        ps[:, lo:hi],
                        kT[pb:pb + 64, 128 * m:128 * (m + 1)],
                        qT[pb:pb + 64, 512 * j + o:512 * (j + 1)],
                        start=True, stop=True)
                pt = ptp.tile([128, 1024], F16,
                              name=f"pt{jp}_{h}_{m}", tag="pt")
                nc.scalar.activation(pt[:, off:], ps[:, off:],
                                     EXP, scale=0.125)
                if jmin in (j0, j1):
                    mo = 512 * (jmin - j0)
                    nc.vector.tensor_mul(
                        pt[:, mo + 128 * d:mo + 512],
                        pt[:, mo + 128 * d:mo + 512],
                        mask[:, 512 * d + 128 * d:512 * (d + 1)])
                pt_tiles[(jp, h, m)] = pt

            def emit_proj_group(qt, n):
                py = acc.tile([128, 512], F32, name=f"yps{qt}_{n}",
                              tag="accps")
                for ks in range(4):
                    nc.tensor.matmul(
                        py[:],
                        ot_sb[ks][:, 128 * qt:128 * (qt + 1)],
                        wp_sb[ks][:, 512 * n:512 * (n + 1)],
                        start=(ks == 0), stop=(ks == 3))
                yb = ybp.tile([128, 512], F32, name=f"yb{qt}_{n}", tag="yb")
                nc.vector.tensor_copy(yb[:], py[:])
                nc.sync.dma_start(
                    y[128 * qt:128 * (qt + 1), 512 * n:512 * (n + 1)], yb[:])

            # ---------------- phase 1: QK^T ---------------------------
            for c in range(CT):
                wts = []
                for k in range(KT):
                    wt = wqkp.tile([128, 128], F16, name=f"wqkt{c}_{k}",
                                   tag="wqkt")
                    nc.sync.dma_start(
                        wt[:], wqk[128 * k:128 * (k + 1), 128 * c:128 * (c + 1)])
                    wts.append(wt)
                if c == 0:
                    # x arrives in column-major 512-chunks so the first
                    # accumulation group is runnable after ~1MB
                    for t in range(TQ):
                        for k in range(KT):
                            nc.sync.dma_start(
                                xt_sb[k][:, 512 * t:512 * (t + 1)],
                                xt[128 * k:128 * (k + 1),
                                   512 * t:512 * (t + 1)])
                for t in range(TQ):
                    pss = acc.tile([128, 512], F32, name=f"qkps{c}_{t}",
                                   tag="accps")
                    for k in range(KT):
                        nc.tensor.matmul(
                            pss[:], wts[k][:],
                            xt_sb[k][:, 512 * t:512 * (t + 1)],
                            start=(k == 0), stop=(k == KT - 1))
                    nc.scalar.copy(qk_sb[c][:, 512 * t:512 * (t + 1)],
                                   pss[:])
                if c == 1:
                    for k in range(KT):
                        nc.sync.dma_start(wv_sb[k][:],
                                          wv[128 * k:128 * (k + 1), :])
                if c == 2:
                    for i in range(4):
                        nc.sync.dma_start(wp_sb[i][:],
                                          wp[128 * i:128 * (i + 1), :])

            # ------- phase 2: V_ext (first half), head-0 prefetch -----
            def emit_v(tt):
                pv = acc.tile([128, 512], F32, name=f"vps{tt}", tag="accps")
                for k in range(KT):
                    nc.tensor.matmul(
                        pv[:], xt_sb[k][:, 128 * tt:128 * (tt + 1)],
                        wv_sb[k][:], start=(k == 0), stop=(k == KT - 1))
                nc.vector.memset(vext[tt].bitcast(mybir.dt.uint16), 0x3C00)
                vdst = vext[tt].rearrange("p (h w) -> p h w", h=HPG)
                nc.vector.tensor_copy(
                    vdst[:, :, 0:HD],
                    pv[:].rearrange("p (h w) -> p h w", h=HPG))

            for tt in range(TT // 2):
                emit_v(tt)
                if tt >= 1:
                    emit_su(0, 0, tt - 1)            # head-0 m 0..6

            # ---------------- phase 3: attention ----------------------
            proj_ready = []
            for jp in range(2):
                j0, j1 = 2 * jp, 2 * jp + 1
                mmax = 8 * jp + 8
                for h in range(HPG):
                    pb = 64 * (h % 2)
                    po = {j: acc.tile([65, 512], F32,
                                      name=f"po{jp}_{h}_{j}", tag="accps")
                          for j in (j0, j1)}
                    for mm in (0, 1):
                        if (jp, h, mm) not in pt_tiles:
                            emit_su(jp, h, mm)
                    for m2 in range(0, mmax, 2):
                      for mm in (m2 + 2, m2 + 3):
                        if mm < mmax and (jp, h, mm) not in pt_tiles:
                            emit_su(jp, h, mm)
                      for m in (m2, m2 + 1):
                        d = m % 4
                        jmin = m // 4
                        pt = pt_tiles.pop((jp, h, m))
                        for j in (j0, j1):
                            if j < jmin:
                                continue
                            o = 128 * d if j == jmin else 0
                            nc.tensor.matmul(
                                po[j][:, o:],
                                vext[m][:, (HD + 1) * h:(HD + 1) * (h + 1)],
                                pt[:, 512 * (j - j0) + o:512 * (j - j0 + 1)],
                                start=(m == 0), stop=(m == 4 * j + 3))
                        if d == 3 and jmin in (j0, j1):
                            j = jmin
                            rs = rsp.tile([1, 512], F32,
                                          name=f"rs{jp}_{h}_{j}", tag="rs")
                            nc.vector.tensor_copy(rs[:], po[j][64:65, :])
                            rc = rsp.tile([1, 512], F32,
                                          name=f"rc{jp}_{h}_{j}", tag="rc")
                            nc.vector.reciprocal_approx_fast(out=rc[:],
                                                             in_=rs[:])
                            rb = rbp.tile([64, 512], F32,
                                          name=f"rb{jp}_{h}_{j}", tag="rb")
                            nc.gpsimd.partition_broadcast(rb[:], rc[:])
                            nc.vector.tensor_mul(
                                ot_sb[h // 2][pb:pb + 64,
                                              512 * j:512 * (j + 1)],
                                po[j][0:64, :], rb[:])
                    # head boundary: filler work for the PE
                    if jp == 0:
                        emit_v(TT // 2 + h)          # V_ext second half
                    for _ in range(2):
                        if proj_ready:
                            emit_proj_group(*proj_ready.pop(0))
                # chunk pair done for all heads -> projection ready
                proj_ready += [(qt, n) for qt in range(8 * jp, 8 * jp + 8)
                               for n in range(C // 512)]
            while proj_ready:
                emit_proj_group(*proj_ready.pop(0))

    nc.compile()
    return nc


def _get_prog():
    if "nc" not in _PROG:
        _PROG["nc"] = _build()
    return _PROG["nc"]


def make_in_maps(x, W_attn, W_proj):
    x = np.asarray(x, dtype=np.float32)
    W_attn = np.asarray(W_attn, dtype=np.float32)
    W_proj = np.asarray(W_proj, dtype=np.float32)
    f16 = np.float16
    in_maps = []
    for core in range(N_CORES):
        b, g = core // 2, core % 2
        in_maps.append({
            "xt": np.ascontiguousarray(x[b].T).astype(f16),
            "wqk": np.ascontiguousarray(np.concatenate(
                [W_attn[:, GC * g:GC * (g + 1)],
                 W_attn[:, C + GC * g:C + GC * (g + 1)]], axis=1)).astype(f16),
            "wv": np.ascontiguousarray(
                W_attn[:, 2 * C + GC * g:2 * C + GC * (g + 1)]).astype(f16),
            "wp": np.ascontiguousarray(
                W_proj[GC * g:GC * (g + 1), :]).astype(f16),
        })
    return in_maps


def run_spmd(in_maps, **kw):
    from concourse.bass_utils import run_bass_kernel_spmd
    return run_bass_kernel_spmd(_get_prog(), in_maps, list(range(N_CORES)), **kw)


def gather(results):
    out = np.empty((B, T, C), np.float32)
    for b in range(B):
        out[b] = results[2 * b]["y"] + results[2 * b + 1]["y"]
    return out


def kernel(x, W_attn, W_proj):
    res = run_spmd(make_in_maps(x, W_attn, W_proj))
    return gather(res.results)
